# revision 36
# baseline (speedup 1.0000x reference)
"""GraphConv GNN kernel for trn2: host preprocessing + bass program builder.

Sharding: nodes (and incident edges, by dst) across 8 cores. Aggregation via
dma_gather (node-major bf16 rows) + one-hot matmul segment-sum. Weights
replicated. Per-layer AllGather of node features, chunked into 4 dst-quarter
blocks so the transfers pipeline with the dense phase. Pooled partial sums +
head computed per-core, summed on host.

Key scheduling facts this kernel is built around:
- dma_gather descriptor generation runs on ONE Q7 core pair selected by
  queue_num (ucode: cpu_id/2 == queue_num), ~8ns/idx. Four SWDGE queues =
  four concurrent pairs: lo-stream ops alternate queues 0/1, hi 2/3.
- Each AllGather chunk is a contiguous row block of hf; windows are packed
  within fixed dst-quarters [0,1563,3125,4688,6250) of each core so the
  lo/hi gather split (int16 idx range) is the fixed predicate
  (dst % npc) < 3125, independent of window packing.
- AG chunk q is triggered as soon as its quarter's dense+transpose+staging
  are issued; chunk q3 is issued at the head of the NEXT layer's gather
  stream (all next-layer gathers depend on it) to avoid Pool-sequencer
  head-of-line deadlock.
"""

import sys

sys.path.insert(0, "/opt/trn_rl_repo")

import numpy as np
import ml_dtypes

import concourse.bass as bass
import concourse.bacc as bacc
import concourse.tile as tile
import concourse.mybir as mybir
from concourse import library_config

BF16 = mybir.dt.bfloat16
F32 = mybir.dt.float32
I16 = mybir.dt.int16

N_CORES = 8
F = 128
N_CLASSES = 10
NQ = 2  # AllGather chunks per layer (dst-halves)

# per-window structure: K_LO lo-chunks + K_HI hi-chunks of 128 edges each
K_LO = 6
K_HI = 6
EDGES_PER_HALF = K_LO * 128  # 768
CHUNKS_PER_WIN = K_LO + K_HI


def _wrap_idx(idx_flat):
    """idx i -> partition i%16, col i//16; replicated across the 8 Q7 core
    stripes (16 partitions each)."""
    n = idx_flat.shape[0]
    return np.ascontiguousarray(
        np.tile(idx_flat.reshape(n // 16, 16).T.astype(np.int16), (8, 1))
    )


def preprocess(x, edge_index, batch, params, n_nodes, n_graphs):
    """Build per-core inputs + meta for the SPMD program."""
    assert n_nodes % N_CORES == 0
    npc = n_nodes // N_CORES
    src = np.asarray(edge_index[0], np.int64)
    dst = np.asarray(edge_index[1], np.int64)
    batch = np.asarray(batch, np.int64)
    x = np.asarray(x, np.float32)

    # fixed dst-half bounds within each core; lo = first half
    qb = [0, npc // 2, npc]
    half_local = qb[1]
    is_lo_node = (np.arange(n_nodes) % npc) < half_local

    # sort edges by dst once
    order = np.argsort(dst, kind="stable")
    src_s, dst_s = src[order], dst[order]

    # per-core edge ranges
    core_edge_start = np.searchsorted(dst_s, np.arange(0, n_nodes + 1, npc))

    # --- pass 1: greedy windows per core per quarter ---
    core_q_windows = [[None] * NQ for _ in range(N_CORES)]
    for k in range(N_CORES):
        e0, e1 = core_edge_start[k], core_edge_start[k + 1]
        dl = dst_s[e0:e1] - k * npc
        sl_lo = is_lo_node[src_s[e0:e1]]
        deg_lo = np.bincount(dl[sl_lo], minlength=npc)
        deg_hi = np.bincount(dl[~sl_lo], minlength=npc)
        for q in range(NQ):
            wins = []
            d = qb[q]
            while d < qb[q + 1]:
                start = d
                lo = hi = 0
                while (
                    d < qb[q + 1]
                    and d - start < 128
                    and lo + deg_lo[d] <= EDGES_PER_HALF
                    and hi + deg_hi[d] <= EDGES_PER_HALF
                ):
                    lo += deg_lo[d]
                    hi += deg_hi[d]
                    d += 1
                assert d > start, "single dst exceeds per-window edge budget"
                wins.append((start, d))
            core_q_windows[k][q] = wins

    W = [max(len(core_q_windows[k][q]) for k in range(N_CORES)) for q in range(NQ)]
    O = [0] * (NQ + 1)
    for q in range(NQ):
        O[q + 1] = O[q] + W[q]
    w_star = O[NQ]
    ls = w_star * 128
    rows = N_CORES * ls
    half_rows = 1024 * W[0]
    assert half_rows <= 32768 and rows - half_rows <= 32768

    # padded global window list per core (empty windows at quarter tails)
    core_windows = []
    for k in range(N_CORES):
        wins = []
        for q in range(NQ):
            wq = core_q_windows[k][q]
            wins.extend(wq)
            wins.extend([(qb[q + 1], qb[q + 1])] * (W[q] - len(wq)))
        core_windows.append(wins)

    # --- slots + rows for every node ---
    slot = np.full(n_nodes, -1, np.int64)
    for k in range(N_CORES):
        for c, (a, b) in enumerate(core_windows[k]):
            if b > a:
                d_loc = np.arange(a, b)
                slot[k * npc + d_loc] = c * 128 + (d_loc - a)
    assert (slot >= 0).all()
    owner = np.arange(n_nodes) // npc
    c_of = slot // 128
    p_of = slot % 128
    q_of = np.searchsorted(np.asarray(O[1:]), c_of, side="right")
    W_arr = np.asarray(W)[q_of]
    O_arr = np.asarray(O)[q_of]
    # block-contiguous per quarter, rank-major inside the block
    row_of = 1024 * O_arr + owner * (128 * W_arr) + p_of * W_arr + (c_of - O_arr)

    # gather op sizing: balance chunks over two queue rounds
    n_chunks = w_star * K_LO
    t_rounds = -(-n_chunks // 64)
    cpo = -(-n_chunks // (2 * t_rounds))
    assert cpo <= 32
    n_ops = -(-n_chunks // cpo)

    # --- per-core streams ---
    per_core = []
    for k in range(N_CORES):
        e0, e1 = core_edge_start[k], core_edge_start[k + 1]
        dl = dst_s[e0:e1] - k * npc
        sv = src_s[e0:e1]
        is_lo = is_lo_node[sv]
        idx_lo = np.zeros((w_star, EDGES_PER_HALF), np.int64)
        ids_lo = np.full((w_star, EDGES_PER_HALF), -1.0, np.float32)
        idx_hi = np.zeros_like(idx_lo)
        ids_hi = np.full_like(ids_lo, -1.0)
        # edges are dst-sorted; window edge groups are contiguous
        wbounds = np.searchsorted(dl, [a for a, _ in core_windows[k]] + [npc])
        for w, (a, b) in enumerate(core_windows[k]):
            lo_m = is_lo[wbounds[w] : wbounds[w + 1]]
            e_dst = dl[wbounds[w] : wbounds[w + 1]]
            e_src = sv[wbounds[w] : wbounds[w + 1]]
            for half, m in ((0, lo_m), (1, ~lo_m)):
                r = row_of[e_src[m]] - (0 if half == 0 else half_rows)
                cnt = r.shape[0]
                assert cnt <= EDGES_PER_HALF
                tgt_idx = idx_lo if half == 0 else idx_hi
                tgt_ids = ids_lo if half == 0 else ids_hi
                tgt_idx[w, :cnt] = r
                tgt_ids[w, :cnt] = (e_dst[m] - a).astype(np.float32)

        def _onehot(ids_arr):
            nch = ids_arr.size // 128
            ids_r = ids_arr.reshape(nch, 128)
            oh = (
                ids_r[:, :, None]
                == np.arange(128, dtype=np.float32)[None, None, :]
            )
            return np.ascontiguousarray(
                oh.transpose(1, 0, 2)
                .reshape(128, nch * 128)
                .astype(ml_dtypes.bfloat16)
            )

        per_core.append(
            dict(
                idx_lo=_wrap_idx(idx_lo.reshape(-1)),
                idx_hi=_wrap_idx(idx_hi.reshape(-1)),
                s_lo=_onehot(ids_lo.reshape(-1)),
                s_hi=_onehot(ids_hi.reshape(-1)),
            )
        )

    # --- x in both layouts ---
    x_bf = x.astype(ml_dtypes.bfloat16)
    x_full_nm = np.zeros((rows, F), ml_dtypes.bfloat16)
    x_full_nm[row_of] = x_bf

    in_maps = []
    for k in range(N_CORES):
        g = np.arange(k * npc, (k + 1) * npc)
        x_fm = np.zeros((F, ls), ml_dtypes.bfloat16)
        x_fm[:, slot[g]] = x_bf[g].T
        b_flat = np.full(ls, -1.0, np.float32)
        b_flat[slot[g]] = batch[g].astype(np.float32)
        batch_nm = b_flat.reshape(w_star, 128).T  # [p, c]
        b_onehot = (
            batch_nm[:, :, None] == np.arange(64, dtype=np.float32)[None, None, :]
        )
        b_onehot = np.ascontiguousarray(
            b_onehot.reshape(128, w_star * 64).astype(ml_dtypes.bfloat16)
        )
        m = dict(
            x_fm=x_fm,
            x_full_nm=x_full_nm,
            b_onehot=b_onehot,
            idx_lo=per_core[k]["idx_lo"],
            idx_hi=per_core[k]["idx_hi"],
            s_lo=per_core[k]["s_lo"],
            s_hi=per_core[k]["s_hi"],
            w1relT=np.ascontiguousarray(params["W1_rel"].T.astype(ml_dtypes.bfloat16)),
            w1rootT=np.ascontiguousarray(
                params["W1_root"].T.astype(ml_dtypes.bfloat16)
            ),
            w2relT=np.ascontiguousarray(params["W2_rel"].T.astype(ml_dtypes.bfloat16)),
            w2rootT=np.ascontiguousarray(
                params["W2_root"].T.astype(ml_dtypes.bfloat16)
            ),
            w3relT=np.ascontiguousarray(params["W3_rel"].T.astype(ml_dtypes.bfloat16)),
            w3rootT=np.ascontiguousarray(
                params["W3_root"].T.astype(ml_dtypes.bfloat16)
            ),
            b1=np.ascontiguousarray(params["b1_rel"].astype(np.float32).reshape(F, 1)),
            b2=np.ascontiguousarray(params["b2_rel"].astype(np.float32).reshape(F, 1)),
            b3=np.ascontiguousarray(params["b3_rel"].astype(np.float32).reshape(F, 1)),
            wlinT=np.ascontiguousarray(params["W_lin"].T.astype(np.float32)),
        )
        in_maps.append(m)

    meta = dict(
        w_star=w_star,
        ls=ls,
        rows=rows,
        half_rows=half_rows,
        n_graphs=n_graphs,
        W=tuple(W),
        O=tuple(O),
        cpo=cpo,
        n_ops=n_ops,
    )
    return meta, in_maps


def build_nc(meta, n_graphs_pad=64):
    w_star = meta["w_star"]
    ls = meta["ls"]
    rows = meta["rows"]
    half_rows = meta["half_rows"]
    W = meta["W"]
    O = meta["O"]
    CPO = meta["cpo"]
    n_ops = meta["n_ops"]
    sl_len = w_star * EDGES_PER_HALF  # idxs per stream
    n_chunks = sl_len // 128
    ng = n_graphs_pad

    nc = bacc.Bacc(
        "TRN2",
        target_bir_lowering=False,
        debug=False,
        num_devices=N_CORES,
        num_swdge_queues=4,
    )

    # --- I/O ---
    x_fm_d = nc.dram_tensor("x_fm", [F, ls], BF16, kind="ExternalInput")
    x_full_d = nc.dram_tensor("x_full_nm", [rows, F], BF16, kind="ExternalInput")
    bone_d = nc.dram_tensor("b_onehot", [128, w_star * 64], BF16, kind="ExternalInput")
    idx_d = {
        "lo": nc.dram_tensor("idx_lo", [128, sl_len // 16], I16, kind="ExternalInput"),
        "hi": nc.dram_tensor("idx_hi", [128, sl_len // 16], I16, kind="ExternalInput"),
    }
    s_d = {
        "lo": nc.dram_tensor("s_lo", [128, n_chunks * 128], BF16, kind="ExternalInput"),
        "hi": nc.dram_tensor("s_hi", [128, n_chunks * 128], BF16, kind="ExternalInput"),
    }
    w_d = {}
    for l in (1, 2, 3):
        for p in ("rel", "root"):
            w_d[l, p] = nc.dram_tensor(f"w{l}{p}T", [F, F], BF16, kind="ExternalInput")
    b_d = {l: nc.dram_tensor(f"b{l}", [F, 1], F32, kind="ExternalInput") for l in (1, 2, 3)}
    wlin_d = nc.dram_tensor("wlinT", [F, N_CLASSES], F32, kind="ExternalInput")
    out_d = nc.dram_tensor("out_partial", [N_CLASSES, ng], F32, kind="ExternalOutput")

    relu = mybir.ActivationFunctionType.Relu
    ident = mybir.ActivationFunctionType.Identity
    copy_f = mybir.ActivationFunctionType.Copy

    with tile.TileContext(nc) as tc:
        with (
            tc.tile_pool(name="const", bufs=1) as constp,
            tc.tile_pool(name="state", bufs=1) as statep,
            tc.tile_pool(name="gpool", bufs=4) as gpool,
            tc.tile_pool(name="spool", bufs=8) as spool,
            tc.tile_pool(name="psa", bufs=4, space="PSUM") as psa,
            tc.tile_pool(name="psd", bufs=2, space="PSUM") as psd,
            tc.tile_pool(name="psp", bufs=1, space="PSUM") as psp,
            tc.tile_pool(name="dram", bufs=1, space="DRAM") as dramp,
        ):
            nc.gpsimd.load_library(library_config.mlp)

            # ---- load constants ----
            bone_t = constp.tile([128, w_star * 64], BF16)
            nc.sync.dma_start(bone_t[:], bone_d[:])
            idx_t = {}
            for h in ("lo", "hi"):
                it = constp.tile([128, sl_len // 16], I16, name=f"idx_{h}")
                nc.sync.dma_start(it[:], idx_d[h][:])
                idx_t[h] = it
            w_t = {}
            for key, d in w_d.items():
                wt = constp.tile([F, F], BF16, name=f"w_{key[0]}_{key[1]}")
                nc.sync.dma_start(wt[:], d[:])
                w_t[key] = wt
            b_t = {}
            for l, d in b_d.items():
                bt = constp.tile([F, 1], F32, name=f"b_{l}")
                nc.sync.dma_start(bt[:], d[:])
                b_t[l] = bt
            wlin_t = constp.tile([F, N_CLASSES], F32)
            nc.sync.dma_start(wlin_t[:], wlin_d[:])

            x_fm_t = statep.tile([F, ls], BF16, tag="h0")
            nc.sync.dma_start(x_fm_t[:], x_fm_d[:])

            # ---- layers ----
            h_fm = x_fm_t
            gather_src = {
                "lo": x_full_d[0:half_rows, :],
                "hi": x_full_d[half_rows:rows, :],
            }
            pending_trigger = None
            for layer in (1, 2, 3):
                g_tiles = {"lo": [None] * n_ops, "hi": [None] * n_ops}
                s_tiles = {"lo": [None] * n_ops, "hi": [None] * n_ops}

                def issue_sload(h, o, layer=layer, s_tiles=s_tiles):
                    c0 = o * CPO
                    c1 = min(n_chunks, c0 + CPO)
                    nch = c1 - c0
                    st_ = spool.tile(
                        [128, nch * 128],
                        BF16,
                        name=f"sg_{layer}_{h}_{o}",
                        tag=f"sg_{h}",
                        bufs=4,
                    )
                    # s-loads go through the SWDGE (Pool) so the HWDGE
                    # semaphore lane carries ONLY the transposes/staging:
                    # Tile's DMA-lane sem targets are cumulative in program
                    # order, so an s-load issued after a transpose would drag
                    # every consumer into waiting for the transpose too.
                    nc.gpsimd.dma_start(st_[:], s_d[h][:, c0 * 128 : c1 * 128])
                    s_tiles[h][o] = st_

                def issue_gather(h, o, layer=layer, g_tiles=g_tiles,
                                 gather_src=gather_src):
                    c0 = o * CPO
                    c1 = min(n_chunks, c0 + CPO)
                    nch = c1 - c0
                    gt = gpool.tile(
                        [128, nch, F],
                        BF16,
                        name=f"g_{layer}_{h}_{o}",
                        tag=f"g_{h}",
                        padded_shape=[128, CPO, F],
                    )
                    nidx = nch * 128
                    # spread gathers over the 4 SWDGE queues: each queue is
                    # served by a dedicated Q7 core pair, so lo ops alternate
                    # pairs 0/1 and hi ops pairs 2/3.
                    qn = (o % 2) if h == "lo" else 2 + (o % 2)
                    nc.gpsimd.dma_gather(
                        gt[:],
                        gather_src[h],
                        idx_t[h][:, c0 * 8 : c1 * 8],
                        nidx,
                        nidx,
                        F,
                        single_packet=False,
                        queue_num=qn,
                    )
                    g_tiles[h][o] = gt

                agg_fm = statep.tile(
                    [F, ls], BF16, tag="agg", name=f"agg_{layer}"
                )
                h_next = statep.tile(
                    [F, ls], BF16, tag=f"h{layer % 2}", name=f"h_{layer}"
                )
                h_nm = statep.tile(
                    [128, w_star, F], BF16, tag="hnm", name=f"hnm_{layer}"
                )
                # two AG chunks: lo half (windows O[0]..O[1]) and hi half.
                # Each half is its own Shared DRAM tensor with exactly one
                # writer (its collective), which keeps the fast Shared-output
                # CC path; the gather streams read the halves separately.
                HB = (0, O[1], O[2])
                if layer < 3:
                    ag_in = [
                        dramp.tile(
                            [128, (HB[i + 1] - HB[i]) * F],
                            BF16,
                            name=f"agin_{layer}_{i}",
                            tag=f"agin{layer % 2}h{i}",
                        )
                        for i in range(2)
                    ]
                    hf = [
                        dramp.tile(
                            [1024 * (HB[i + 1] - HB[i]), F],
                            BF16,
                            name=f"hf_{layer}_{i}",
                            tag=f"hf{layer}h{i}",
                            addr_space="Shared",
                        )
                        for i in range(2)
                    ]

                s_issued = [0]

                def quarter_block(q, layer=layer, g_tiles=g_tiles, s_tiles=s_tiles,
                                  agg_fm=agg_fm, h_next=h_next, h_nm=h_nm,
                                  h_fm=h_fm, s_issued=s_issued):
                    for w in range(O[q], O[q + 1]):
                        ps = psa.tile(
                            [128, 128], F32, name=f"psagg_{layer}_{w}", tag="psagg"
                        )
                        for j in range(CHUNKS_PER_WIN):
                            h = "lo" if j < K_LO else "hi"
                            cc = w * K_LO + (j % K_LO)
                            o, sl_ = cc // CPO, cc % CPO
                            nc.tensor.matmul(
                                ps[:],
                                g_tiles[h][o][:, sl_, :],
                                s_tiles[h][o][:, sl_ * 128 : (sl_ + 1) * 128],
                                start=(j == 0),
                                stop=(j == CHUNKS_PER_WIN - 1),
                            )
                        nc.scalar.activation(
                            agg_fm[:, w * 128 : (w + 1) * 128], ps[:], copy_f
                        )

                    # dense over this quarter's columns
                    c0_ = O[q] * 128
                    cq1 = O[q + 1] * 128
                    while c0_ < cq1:
                        cw = min(512, cq1 - c0_)
                        ps = psd.tile(
                            [128, 512], F32, name=f"psd_{layer}_{c0_}", tag="psd"
                        )
                        sl2 = slice(c0_, c0_ + cw)
                        nc.tensor.matmul(
                            ps[:, :cw],
                            w_t[layer, "rel"][:],
                            agg_fm[:, sl2],
                            start=True,
                            stop=False,
                        )
                        nc.tensor.matmul(
                            ps[:, :cw],
                            w_t[layer, "root"][:],
                            h_fm[:, sl2],
                            start=False,
                            stop=True,
                        )
                        nc.scalar.activation(
                            h_next[:, sl2],
                            ps[:, :cw],
                            relu if layer < 3 else ident,
                            bias=b_t[layer][:],
                        )
                        c0_ += cw

                def share_block(i, h_next=h_next, h_nm=h_nm, HB=HB,
                                ag_in=ag_in if layer < 3 else None,
                                layer=layer):
                    nc.sync.dma_start_transpose(
                        h_nm[:, HB[i] : HB[i + 1], :],
                        h_next[:, HB[i] * 128 : HB[i + 1] * 128],
                    )
                    if layer < 3:
                        nc.sync.dma_start(
                            ag_in[i][:], h_nm[:, HB[i] : HB[i + 1], :]
                        )

                def issue_ag(i, ag_in=ag_in if layer < 3 else None,
                             hf=hf if layer < 3 else None):
                    nc.gpsimd.collective_compute(
                        "AllGather",
                        mybir.AluOpType.bypass,
                        replica_groups=[list(range(N_CORES))],
                        ins=[ag_in[i][:]],
                        outs=[hf[i][:]],
                    )

                # Issue sequence per layer:
                #   1. all gather ops (Pool stays pure; pacing via buffer
                #      rings) — s-loads ride Scalar at quarter boundaries;
                #   2. compute in two half-blocks so the lo half's dense +
                #      transpose + staging complete while the hi half still
                #      aggregates, letting AG-lo's transfer overlap it;
                #   3. AG triggers (Pool, after the gathers) fire as their
                #      staging lands; next layer's lo gathers only wait on
                #      AG-lo, so they start one transfer earlier than hi.
                for o in range(n_ops):
                    issue_sload("lo", o)
                    issue_sload("hi", o)
                    issue_gather("lo", o)
                    issue_gather("hi", o)
                quarter_block(0)
                share_block(0)
                if layer < 3:
                    issue_ag(0)
                quarter_block(1)
                share_block(1)
                if layer < 3:
                    issue_ag(1)

                if layer < 3:
                    gather_src = {
                        "lo": hf[0][:, :],
                        "hi": hf[1][:, :],
                    }
                    h_fm = h_next
                else:
                    # pooling: pooledT[f, g] += h_nm[:, c, :].T @ B
                    ps_pool = psp.tile([128, ng], F32, tag="pspool")
                    for c in range(w_star):
                        nc.tensor.matmul(
                            ps_pool[:],
                            h_nm[:, c, :],
                            bone_t[:, c * ng : (c + 1) * ng],
                            start=(c == 0),
                            stop=(c == w_star - 1),
                        )
                    pooledT = statep.tile([128, ng], F32, tag="pooledT")
                    nc.scalar.activation(pooledT[:], ps_pool[:], copy_f)
                    ps_head = psp.tile([N_CLASSES, ng], F32, tag="pshead")
                    nc.tensor.matmul(ps_head[:], wlin_t[:], pooledT[:])
                    out_sb = statep.tile([N_CLASSES, ng], F32, tag="outsb")
                    nc.vector.tensor_copy(out_sb[:], ps_head[:])
                    nc.sync.dma_start(out_d[:], out_sb[:])

    nc.compile()
    return nc


def postprocess(results, batch, b_lin, n_graphs):
    """results: list of per-core dicts with 'out_partial' [10, ng]."""
    total = np.zeros_like(np.asarray(results[0]["out_partial"], np.float32))
    for r in results:
        total += np.asarray(r["out_partial"], np.float32)
    cnt = np.bincount(np.asarray(batch, np.int64), minlength=n_graphs).astype(
        np.float32
    )
    cnt = np.maximum(cnt, 1.0)
    logits = total[:, :n_graphs].T / cnt[:, None] + np.asarray(b_lin, np.float32)[None, :]
    return logits.astype(np.float32)


# ----------------------------------------------------------------------------
# harness entry point
# ----------------------------------------------------------------------------
from concourse.bass_utils import run_bass_kernel_spmd

_CACHE = {}


def kernel(x, edge_index, batch,
           W1_rel, b1_rel, W1_root,
           W2_rel, b2_rel, W2_root,
           W3_rel, b3_rel, W3_root,
           W_lin, b_lin):
    params = dict(W1_rel=W1_rel, b1_rel=b1_rel, W1_root=W1_root,
                  W2_rel=W2_rel, b2_rel=b2_rel, W2_root=W2_root,
                  W3_rel=W3_rel, b3_rel=b3_rel, W3_root=W3_root,
                  W_lin=W_lin, b_lin=b_lin)
    n_nodes = int(np.asarray(x).shape[0])
    n_graphs = 64
    meta, in_maps = preprocess(x, edge_index, batch, params, n_nodes, n_graphs)
    key = (meta["w_star"], meta["W"], meta["cpo"])
    if key not in _CACHE:
        _CACHE[key] = build_nc(meta)
    nc = _CACHE[key]
    res = run_bass_kernel_spmd(nc, in_maps, core_ids=list(range(N_CORES)))
    return postprocess(res.results, batch, b_lin, n_graphs)


# revision 37
# speedup vs baseline: 1.1083x; 1.1083x over previous
"""GraphConv GNN kernel for trn2: host preprocessing + bass program builder.

Sharding: nodes (and incident edges, by dst) across 8 cores. Aggregation via
dma_gather (node-major bf16 rows) + one-hot matmul segment-sum. Weights
replicated. Per-layer AllGather of node features, chunked into 4 dst-quarter
blocks so the transfers pipeline with the dense phase. Pooled partial sums +
head computed per-core, summed on host.

Key scheduling facts this kernel is built around:
- dma_gather descriptor generation runs on ONE Q7 core pair selected by
  queue_num (ucode: cpu_id/2 == queue_num), ~8ns/idx. Four SWDGE queues =
  four concurrent pairs: lo-stream ops alternate queues 0/1, hi 2/3.
- Each AllGather chunk is a contiguous row block of hf; windows are packed
  within fixed dst-quarters [0,1563,3125,4688,6250) of each core so the
  lo/hi gather split (int16 idx range) is the fixed predicate
  (dst % npc) < 3125, independent of window packing.
- AG chunk q is triggered as soon as its quarter's dense+transpose+staging
  are issued; chunk q3 is issued at the head of the NEXT layer's gather
  stream (all next-layer gathers depend on it) to avoid Pool-sequencer
  head-of-line deadlock.
"""

import sys

sys.path.insert(0, "/opt/trn_rl_repo")

import numpy as np
import ml_dtypes

import concourse.bass as bass
import concourse.bacc as bacc
import concourse.tile as tile
import concourse.mybir as mybir
from concourse import library_config

BF16 = mybir.dt.bfloat16
F32 = mybir.dt.float32
I16 = mybir.dt.int16

N_CORES = 8
F = 128
N_CLASSES = 10
NQ = 2  # AllGather chunks per layer (dst-halves)

# per-window structure: K_LO lo-chunks + K_HI hi-chunks of 128 edges each
K_LO = 6
K_HI = 6
EDGES_PER_HALF = K_LO * 128  # 768
CHUNKS_PER_WIN = K_LO + K_HI


def _wrap_idx(idx_flat):
    """idx i -> partition i%16, col i//16; replicated across the 8 Q7 core
    stripes (16 partitions each)."""
    n = idx_flat.shape[0]
    return np.ascontiguousarray(
        np.tile(idx_flat.reshape(n // 16, 16).T.astype(np.int16), (8, 1))
    )


def preprocess(x, edge_index, batch, params, n_nodes, n_graphs):
    """Build per-core inputs + meta for the SPMD program."""
    assert n_nodes % N_CORES == 0
    npc = n_nodes // N_CORES
    src = np.asarray(edge_index[0], np.int64)
    dst = np.asarray(edge_index[1], np.int64)
    batch = np.asarray(batch, np.int64)
    x = np.asarray(x, np.float32)

    # fixed dst-half bounds within each core; lo = first half
    qb = [0, npc // 2, npc]
    half_local = qb[1]
    is_lo_node = (np.arange(n_nodes) % npc) < half_local

    # sort edges by dst once
    order = np.argsort(dst, kind="stable")
    src_s, dst_s = src[order], dst[order]

    # per-core edge ranges
    core_edge_start = np.searchsorted(dst_s, np.arange(0, n_nodes + 1, npc))

    # --- pass 1: greedy windows per core per quarter ---
    core_q_windows = [[None] * NQ for _ in range(N_CORES)]
    for k in range(N_CORES):
        e0, e1 = core_edge_start[k], core_edge_start[k + 1]
        dl = dst_s[e0:e1] - k * npc
        sl_lo = is_lo_node[src_s[e0:e1]]
        deg_lo = np.bincount(dl[sl_lo], minlength=npc)
        deg_hi = np.bincount(dl[~sl_lo], minlength=npc)
        for q in range(NQ):
            wins = []
            d = qb[q]
            while d < qb[q + 1]:
                start = d
                lo = hi = 0
                while (
                    d < qb[q + 1]
                    and d - start < 128
                    and lo + deg_lo[d] <= EDGES_PER_HALF
                    and hi + deg_hi[d] <= EDGES_PER_HALF
                ):
                    lo += deg_lo[d]
                    hi += deg_hi[d]
                    d += 1
                assert d > start, "single dst exceeds per-window edge budget"
                wins.append((start, d))
            core_q_windows[k][q] = wins

    W = [max(len(core_q_windows[k][q]) for k in range(N_CORES)) for q in range(NQ)]
    O = [0] * (NQ + 1)
    for q in range(NQ):
        O[q + 1] = O[q] + W[q]
    w_star = O[NQ]
    ls = w_star * 128
    rows = N_CORES * ls
    half_rows = 1024 * W[0]
    assert half_rows <= 32768 and rows - half_rows <= 32768

    # padded global window list per core (empty windows at quarter tails)
    core_windows = []
    for k in range(N_CORES):
        wins = []
        for q in range(NQ):
            wq = core_q_windows[k][q]
            wins.extend(wq)
            wins.extend([(qb[q + 1], qb[q + 1])] * (W[q] - len(wq)))
        core_windows.append(wins)

    # --- slots + rows for every node ---
    slot = np.full(n_nodes, -1, np.int64)
    for k in range(N_CORES):
        for c, (a, b) in enumerate(core_windows[k]):
            if b > a:
                d_loc = np.arange(a, b)
                slot[k * npc + d_loc] = c * 128 + (d_loc - a)
    assert (slot >= 0).all()
    owner = np.arange(n_nodes) // npc
    c_of = slot // 128
    p_of = slot % 128
    q_of = np.searchsorted(np.asarray(O[1:]), c_of, side="right")
    W_arr = np.asarray(W)[q_of]
    O_arr = np.asarray(O)[q_of]
    # block-contiguous per quarter, rank-major inside the block
    row_of = 1024 * O_arr + owner * (128 * W_arr) + p_of * W_arr + (c_of - O_arr)

    # gather op sizing: balance chunks over two queue rounds
    n_chunks = w_star * K_LO
    t_rounds = -(-n_chunks // 64)
    cpo = -(-n_chunks // (2 * t_rounds))
    assert cpo <= 32
    n_ops = -(-n_chunks // cpo)

    # --- per-core streams ---
    per_core = []
    for k in range(N_CORES):
        e0, e1 = core_edge_start[k], core_edge_start[k + 1]
        dl = dst_s[e0:e1] - k * npc
        sv = src_s[e0:e1]
        is_lo = is_lo_node[sv]
        idx_lo = np.zeros((w_star, EDGES_PER_HALF), np.int64)
        ids_lo = np.full((w_star, EDGES_PER_HALF), -1.0, np.float32)
        idx_hi = np.zeros_like(idx_lo)
        ids_hi = np.full_like(ids_lo, -1.0)
        # edges are dst-sorted; window edge groups are contiguous
        wbounds = np.searchsorted(dl, [a for a, _ in core_windows[k]] + [npc])
        for w, (a, b) in enumerate(core_windows[k]):
            lo_m = is_lo[wbounds[w] : wbounds[w + 1]]
            e_dst = dl[wbounds[w] : wbounds[w + 1]]
            e_src = sv[wbounds[w] : wbounds[w + 1]]
            for half, m in ((0, lo_m), (1, ~lo_m)):
                r = row_of[e_src[m]] - (0 if half == 0 else half_rows)
                cnt = r.shape[0]
                assert cnt <= EDGES_PER_HALF
                tgt_idx = idx_lo if half == 0 else idx_hi
                tgt_ids = ids_lo if half == 0 else ids_hi
                tgt_idx[w, :cnt] = r
                tgt_ids[w, :cnt] = (e_dst[m] - a).astype(np.float32)

        def _onehot(ids_arr):
            nch = ids_arr.size // 128
            ids_r = ids_arr.reshape(nch, 128)
            oh = (
                ids_r[:, :, None]
                == np.arange(128, dtype=np.float32)[None, None, :]
            )
            return np.ascontiguousarray(
                oh.transpose(1, 0, 2)
                .reshape(128, nch * 128)
                .astype(ml_dtypes.bfloat16)
            )

        per_core.append(
            dict(
                idx_lo=_wrap_idx(idx_lo.reshape(-1)),
                idx_hi=_wrap_idx(idx_hi.reshape(-1)),
                s_lo=_onehot(ids_lo.reshape(-1)),
                s_hi=_onehot(ids_hi.reshape(-1)),
            )
        )

    # --- x in both layouts ---
    x_bf = x.astype(ml_dtypes.bfloat16)
    x_full_nm = np.zeros((rows, F), ml_dtypes.bfloat16)
    x_full_nm[row_of] = x_bf

    in_maps = []
    for k in range(N_CORES):
        g = np.arange(k * npc, (k + 1) * npc)
        x_fm = np.zeros((F, ls), ml_dtypes.bfloat16)
        x_fm[:, slot[g]] = x_bf[g].T
        b_flat = np.full(ls, -1.0, np.float32)
        b_flat[slot[g]] = batch[g].astype(np.float32)
        batch_nm = b_flat.reshape(w_star, 128).T  # [p, c]
        b_onehot = (
            batch_nm[:, :, None] == np.arange(64, dtype=np.float32)[None, None, :]
        )
        b_onehot = np.ascontiguousarray(
            b_onehot.reshape(128, w_star * 64).astype(ml_dtypes.bfloat16)
        )
        m = dict(
            x_fm=x_fm,
            x_full_nm=x_full_nm,
            b_onehot=b_onehot,
            idx_lo=per_core[k]["idx_lo"],
            idx_hi=per_core[k]["idx_hi"],
            s_lo=per_core[k]["s_lo"],
            s_hi=per_core[k]["s_hi"],
            w1relT=np.ascontiguousarray(params["W1_rel"].T.astype(ml_dtypes.bfloat16)),
            w1rootT=np.ascontiguousarray(
                params["W1_root"].T.astype(ml_dtypes.bfloat16)
            ),
            w2relT=np.ascontiguousarray(params["W2_rel"].T.astype(ml_dtypes.bfloat16)),
            w2rootT=np.ascontiguousarray(
                params["W2_root"].T.astype(ml_dtypes.bfloat16)
            ),
            w3relT=np.ascontiguousarray(params["W3_rel"].T.astype(ml_dtypes.bfloat16)),
            w3rootT=np.ascontiguousarray(
                params["W3_root"].T.astype(ml_dtypes.bfloat16)
            ),
            b1=np.ascontiguousarray(params["b1_rel"].astype(np.float32).reshape(F, 1)),
            b2=np.ascontiguousarray(params["b2_rel"].astype(np.float32).reshape(F, 1)),
            b3=np.ascontiguousarray(params["b3_rel"].astype(np.float32).reshape(F, 1)),
            wlinT=np.ascontiguousarray(params["W_lin"].T.astype(np.float32)),
        )
        in_maps.append(m)

    meta = dict(
        w_star=w_star,
        ls=ls,
        rows=rows,
        half_rows=half_rows,
        n_graphs=n_graphs,
        W=tuple(W),
        O=tuple(O),
        cpo=cpo,
        n_ops=n_ops,
    )
    return meta, in_maps


def build_nc(meta, n_graphs_pad=64):
    w_star = meta["w_star"]
    ls = meta["ls"]
    rows = meta["rows"]
    half_rows = meta["half_rows"]
    W = meta["W"]
    O = meta["O"]
    CPO = meta["cpo"]
    n_ops = meta["n_ops"]
    sl_len = w_star * EDGES_PER_HALF  # idxs per stream
    n_chunks = sl_len // 128
    ng = n_graphs_pad

    nc = bacc.Bacc(
        "TRN2",
        target_bir_lowering=False,
        debug=False,
        num_devices=N_CORES,
        num_swdge_queues=4,
    )

    # --- I/O ---
    x_fm_d = nc.dram_tensor("x_fm", [F, ls], BF16, kind="ExternalInput")
    x_full_d = nc.dram_tensor("x_full_nm", [rows, F], BF16, kind="ExternalInput")
    bone_d = nc.dram_tensor("b_onehot", [128, w_star * 64], BF16, kind="ExternalInput")
    idx_d = {
        "lo": nc.dram_tensor("idx_lo", [128, sl_len // 16], I16, kind="ExternalInput"),
        "hi": nc.dram_tensor("idx_hi", [128, sl_len // 16], I16, kind="ExternalInput"),
    }
    s_d = {
        "lo": nc.dram_tensor("s_lo", [128, n_chunks * 128], BF16, kind="ExternalInput"),
        "hi": nc.dram_tensor("s_hi", [128, n_chunks * 128], BF16, kind="ExternalInput"),
    }
    w_d = {}
    for l in (1, 2, 3):
        for p in ("rel", "root"):
            w_d[l, p] = nc.dram_tensor(f"w{l}{p}T", [F, F], BF16, kind="ExternalInput")
    b_d = {l: nc.dram_tensor(f"b{l}", [F, 1], F32, kind="ExternalInput") for l in (1, 2, 3)}
    wlin_d = nc.dram_tensor("wlinT", [F, N_CLASSES], F32, kind="ExternalInput")
    out_d = nc.dram_tensor("out_partial", [N_CLASSES, ng], F32, kind="ExternalOutput")

    relu = mybir.ActivationFunctionType.Relu
    ident = mybir.ActivationFunctionType.Identity
    copy_f = mybir.ActivationFunctionType.Copy

    with tile.TileContext(nc) as tc:
        with (
            tc.tile_pool(name="const", bufs=1) as constp,
            tc.tile_pool(name="state", bufs=1) as statep,
            tc.tile_pool(name="gpool", bufs=4) as gpool,
            tc.tile_pool(name="spool", bufs=8) as spool,
            tc.tile_pool(name="psa", bufs=4, space="PSUM") as psa,
            tc.tile_pool(name="psd", bufs=2, space="PSUM") as psd,
            tc.tile_pool(name="psp", bufs=1, space="PSUM") as psp,
            tc.tile_pool(name="dram", bufs=1, space="DRAM") as dramp,
        ):
            nc.gpsimd.load_library(library_config.mlp)

            # ---- load constants ----
            bone_t = constp.tile([128, w_star * 64], BF16)
            nc.sync.dma_start(bone_t[:], bone_d[:])
            idx_t = {}
            for h in ("lo", "hi"):
                it = constp.tile([128, sl_len // 16], I16, name=f"idx_{h}")
                nc.sync.dma_start(it[:], idx_d[h][:])
                idx_t[h] = it
            w_t = {}
            for key, d in w_d.items():
                wt = constp.tile([F, F], BF16, name=f"w_{key[0]}_{key[1]}")
                nc.sync.dma_start(wt[:], d[:])
                w_t[key] = wt
            b_t = {}
            for l, d in b_d.items():
                bt = constp.tile([F, 1], F32, name=f"b_{l}")
                nc.sync.dma_start(bt[:], d[:])
                b_t[l] = bt
            wlin_t = constp.tile([F, N_CLASSES], F32)
            nc.sync.dma_start(wlin_t[:], wlin_d[:])

            x_fm_t = statep.tile([F, ls], BF16, tag="h0")
            nc.sync.dma_start(x_fm_t[:], x_fm_d[:])

            # ---- layers ----
            h_fm = x_fm_t
            gather_src = {
                "lo": x_full_d[0:half_rows, :],
                "hi": x_full_d[half_rows:rows, :],
            }
            pending_trigger = None
            for layer in (1, 2, 3):
                g_tiles = {"lo": [None] * n_ops, "hi": [None] * n_ops}
                s_tiles = {"lo": [None] * n_ops, "hi": [None] * n_ops}

                def issue_sload(h, o, layer=layer, s_tiles=s_tiles):
                    c0 = o * CPO
                    c1 = min(n_chunks, c0 + CPO)
                    nch = c1 - c0
                    st_ = spool.tile(
                        [128, nch * 128],
                        BF16,
                        name=f"sg_{layer}_{h}_{o}",
                        tag=f"sg_{h}",
                        bufs=4,
                    )
                    # s-loads ride the Sync HWDGE, all issued up-front so
                    # they precede the transposes in the HWDGE semaphore
                    # lane: Tile's DMA-lane sem targets are cumulative in
                    # program order, so an s-load issued after a transpose
                    # would drag all its consumers into waiting for it.
                    nc.sync.dma_start(st_[:], s_d[h][:, c0 * 128 : c1 * 128])
                    s_tiles[h][o] = st_

                def issue_gather(h, o, layer=layer, g_tiles=g_tiles,
                                 gather_src=gather_src):
                    c0 = o * CPO
                    c1 = min(n_chunks, c0 + CPO)
                    nch = c1 - c0
                    gt = gpool.tile(
                        [128, nch, F],
                        BF16,
                        name=f"g_{layer}_{h}_{o}",
                        tag=f"g_{h}",
                        padded_shape=[128, CPO, F],
                    )
                    nidx = nch * 128
                    # spread gathers over the 4 SWDGE queues: each queue is
                    # served by a dedicated Q7 core pair, so lo ops alternate
                    # pairs 0/1 and hi ops pairs 2/3.
                    qn = (o % 2) if h == "lo" else 2 + (o % 2)
                    nc.gpsimd.dma_gather(
                        gt[:],
                        gather_src[h],
                        idx_t[h][:, c0 * 8 : c1 * 8],
                        nidx,
                        nidx,
                        F,
                        single_packet=False,
                        queue_num=qn,
                    )
                    g_tiles[h][o] = gt

                agg_fm = statep.tile(
                    [F, ls], BF16, tag="agg", name=f"agg_{layer}"
                )
                h_next = statep.tile(
                    [F, ls], BF16, tag=f"h{layer % 2}", name=f"h_{layer}"
                )
                h_nm = statep.tile(
                    [128, w_star, F], BF16, tag="hnm", name=f"hnm_{layer}"
                )
                # two AG chunks: lo half (windows O[0]..O[1]) and hi half.
                # Each half is its own Shared DRAM tensor with exactly one
                # writer (its collective), which keeps the fast Shared-output
                # CC path; the gather streams read the halves separately.
                HB = (0, O[1], O[2])
                if layer < 3:
                    ag_in = [
                        dramp.tile(
                            [128, (HB[i + 1] - HB[i]) * F],
                            BF16,
                            name=f"agin_{layer}_{i}",
                            tag=f"agin{layer % 2}h{i}",
                        )
                        for i in range(2)
                    ]
                    hf = [
                        dramp.tile(
                            [1024 * (HB[i + 1] - HB[i]), F],
                            BF16,
                            name=f"hf_{layer}_{i}",
                            tag=f"hf{layer}h{i}",
                            addr_space="Shared",
                        )
                        for i in range(2)
                    ]

                s_issued = [0]

                def quarter_block(q, layer=layer, g_tiles=g_tiles, s_tiles=s_tiles,
                                  agg_fm=agg_fm, h_next=h_next, h_nm=h_nm,
                                  h_fm=h_fm, s_issued=s_issued):
                    for w in range(O[q], O[q + 1]):
                        ps = psa.tile(
                            [128, 128], F32, name=f"psagg_{layer}_{w}", tag="psagg"
                        )
                        for j in range(CHUNKS_PER_WIN):
                            h = "lo" if j < K_LO else "hi"
                            cc = w * K_LO + (j % K_LO)
                            o, sl_ = cc // CPO, cc % CPO
                            nc.tensor.matmul(
                                ps[:],
                                g_tiles[h][o][:, sl_, :],
                                s_tiles[h][o][:, sl_ * 128 : (sl_ + 1) * 128],
                                start=(j == 0),
                                stop=(j == CHUNKS_PER_WIN - 1),
                            )
                        nc.scalar.activation(
                            agg_fm[:, w * 128 : (w + 1) * 128], ps[:], copy_f
                        )

                    # dense over this quarter's columns
                    c0_ = O[q] * 128
                    cq1 = O[q + 1] * 128
                    while c0_ < cq1:
                        cw = min(512, cq1 - c0_)
                        ps = psd.tile(
                            [128, 512], F32, name=f"psd_{layer}_{c0_}", tag="psd"
                        )
                        sl2 = slice(c0_, c0_ + cw)
                        nc.tensor.matmul(
                            ps[:, :cw],
                            w_t[layer, "rel"][:],
                            agg_fm[:, sl2],
                            start=True,
                            stop=False,
                        )
                        nc.tensor.matmul(
                            ps[:, :cw],
                            w_t[layer, "root"][:],
                            h_fm[:, sl2],
                            start=False,
                            stop=True,
                        )
                        nc.scalar.activation(
                            h_next[:, sl2],
                            ps[:, :cw],
                            relu if layer < 3 else ident,
                            bias=b_t[layer][:],
                        )
                        c0_ += cw

                def share_block(i, h_next=h_next, h_nm=h_nm, HB=HB,
                                ag_in=ag_in if layer < 3 else None,
                                layer=layer):
                    nc.sync.dma_start_transpose(
                        h_nm[:, HB[i] : HB[i + 1], :],
                        h_next[:, HB[i] * 128 : HB[i + 1] * 128],
                    )
                    if layer < 3:
                        nc.sync.dma_start(
                            ag_in[i][:], h_nm[:, HB[i] : HB[i + 1], :]
                        )

                def issue_ag(i, ag_in=ag_in if layer < 3 else None,
                             hf=hf if layer < 3 else None):
                    nc.gpsimd.collective_compute(
                        "AllGather",
                        mybir.AluOpType.bypass,
                        replica_groups=[list(range(N_CORES))],
                        ins=[ag_in[i][:]],
                        outs=[hf[i][:]],
                    )

                # Issue sequence per layer:
                #   1. all gather ops (Pool stays pure; pacing via buffer
                #      rings) — s-loads ride Scalar at quarter boundaries;
                #   2. compute in two half-blocks so the lo half's dense +
                #      transpose + staging complete while the hi half still
                #      aggregates, letting AG-lo's transfer overlap it;
                #   3. AG triggers (Pool, after the gathers) fire as their
                #      staging lands; next layer's lo gathers only wait on
                #      AG-lo, so they start one transfer earlier than hi.
                for o in range(n_ops):
                    issue_sload("lo", o)
                    issue_sload("hi", o)
                    issue_gather("lo", o)
                    issue_gather("hi", o)
                quarter_block(0)
                share_block(0)
                if layer < 3:
                    issue_ag(0)
                quarter_block(1)
                share_block(1)
                if layer < 3:
                    issue_ag(1)

                if layer < 3:
                    gather_src = {
                        "lo": hf[0][:, :],
                        "hi": hf[1][:, :],
                    }
                    h_fm = h_next
                else:
                    # pooling: pooledT[f, g] += h_nm[:, c, :].T @ B
                    ps_pool = psp.tile([128, ng], F32, tag="pspool")
                    for c in range(w_star):
                        nc.tensor.matmul(
                            ps_pool[:],
                            h_nm[:, c, :],
                            bone_t[:, c * ng : (c + 1) * ng],
                            start=(c == 0),
                            stop=(c == w_star - 1),
                        )
                    pooledT = statep.tile([128, ng], F32, tag="pooledT")
                    nc.scalar.activation(pooledT[:], ps_pool[:], copy_f)
                    ps_head = psp.tile([N_CLASSES, ng], F32, tag="pshead")
                    nc.tensor.matmul(ps_head[:], wlin_t[:], pooledT[:])
                    out_sb = statep.tile([N_CLASSES, ng], F32, tag="outsb")
                    nc.vector.tensor_copy(out_sb[:], ps_head[:])
                    nc.sync.dma_start(out_d[:], out_sb[:])

    nc.compile()
    return nc


def postprocess(results, batch, b_lin, n_graphs):
    """results: list of per-core dicts with 'out_partial' [10, ng]."""
    total = np.zeros_like(np.asarray(results[0]["out_partial"], np.float32))
    for r in results:
        total += np.asarray(r["out_partial"], np.float32)
    cnt = np.bincount(np.asarray(batch, np.int64), minlength=n_graphs).astype(
        np.float32
    )
    cnt = np.maximum(cnt, 1.0)
    logits = total[:, :n_graphs].T / cnt[:, None] + np.asarray(b_lin, np.float32)[None, :]
    return logits.astype(np.float32)


# ----------------------------------------------------------------------------
# harness entry point
# ----------------------------------------------------------------------------
from concourse.bass_utils import run_bass_kernel_spmd

_CACHE = {}


def kernel(x, edge_index, batch,
           W1_rel, b1_rel, W1_root,
           W2_rel, b2_rel, W2_root,
           W3_rel, b3_rel, W3_root,
           W_lin, b_lin):
    params = dict(W1_rel=W1_rel, b1_rel=b1_rel, W1_root=W1_root,
                  W2_rel=W2_rel, b2_rel=b2_rel, W2_root=W2_root,
                  W3_rel=W3_rel, b3_rel=b3_rel, W3_root=W3_root,
                  W_lin=W_lin, b_lin=b_lin)
    n_nodes = int(np.asarray(x).shape[0])
    n_graphs = 64
    meta, in_maps = preprocess(x, edge_index, batch, params, n_nodes, n_graphs)
    key = (meta["w_star"], meta["W"], meta["cpo"])
    if key not in _CACHE:
        _CACHE[key] = build_nc(meta)
    nc = _CACHE[key]
    res = run_bass_kernel_spmd(nc, in_maps, core_ids=list(range(N_CORES)))
    return postprocess(res.results, batch, b_lin, n_graphs)


# revision 43
# speedup vs baseline: 1.2553x; 1.1326x over previous
"""GraphConv GNN kernel for trn2: host preprocessing + bass program builder.

Sharding: nodes (and incident edges, by dst) across 8 cores. Aggregation via
dma_gather (node-major bf16 rows) + one-hot matmul segment-sum. Weights
replicated. Per-layer AllGather of node features, chunked into 4 dst-quarter
blocks so the transfers pipeline with the dense phase. Pooled partial sums +
head computed per-core, summed on host.

Key scheduling facts this kernel is built around:
- dma_gather descriptor generation runs on ONE Q7 core pair selected by
  queue_num (ucode: cpu_id/2 == queue_num), ~8ns/idx. Four SWDGE queues =
  four concurrent pairs: lo-stream ops alternate queues 0/1, hi 2/3.
- Each AllGather chunk is a contiguous row block of hf; windows are packed
  within fixed dst-quarters [0,1563,3125,4688,6250) of each core so the
  lo/hi gather split (int16 idx range) is the fixed predicate
  (dst % npc) < 3125, independent of window packing.
- AG chunk q is triggered as soon as its quarter's dense+transpose+staging
  are issued; chunk q3 is issued at the head of the NEXT layer's gather
  stream (all next-layer gathers depend on it) to avoid Pool-sequencer
  head-of-line deadlock.
"""

import sys

sys.path.insert(0, "/opt/trn_rl_repo")

import numpy as np
import ml_dtypes

import concourse.bass as bass
import concourse.bacc as bacc
import concourse.tile as tile
import concourse.mybir as mybir
from concourse import library_config

BF16 = mybir.dt.bfloat16
F32 = mybir.dt.float32
I16 = mybir.dt.int16

N_CORES = 8
F = 128
N_CLASSES = 10
NQ = 2  # AllGather chunks per layer (dst-halves)

# per-window structure: K_LO lo-chunks + K_HI hi-chunks of 128 edges each
K_LO = 6
K_HI = 6
EDGES_PER_HALF = K_LO * 128  # 768
CHUNKS_PER_WIN = K_LO + K_HI


def _wrap_idx(idx_flat):
    """idx i -> partition i%16, col i//16; replicated across the 8 Q7 core
    stripes (16 partitions each)."""
    n = idx_flat.shape[0]
    return np.ascontiguousarray(
        np.tile(idx_flat.reshape(n // 16, 16).T.astype(np.int16), (8, 1))
    )


def preprocess(x, edge_index, batch, params, n_nodes, n_graphs):
    """Build per-core inputs + meta for the SPMD program."""
    assert n_nodes % N_CORES == 0
    npc = n_nodes // N_CORES
    src = np.asarray(edge_index[0], np.int64)
    dst = np.asarray(edge_index[1], np.int64)
    batch = np.asarray(batch, np.int64)
    x = np.asarray(x, np.float32)

    # owner-based lo/hi split: rows are owner-major (single AllGather
    # concatenates whole per-core shards), so lo = cores 0-3
    half_node = (N_CORES // 2) * npc
    is_lo_node = np.arange(n_nodes) < half_node

    # sort edges by dst once
    order = np.argsort(dst, kind="stable")
    src_s, dst_s = src[order], dst[order]

    # per-core edge ranges
    core_edge_start = np.searchsorted(dst_s, np.arange(0, n_nodes + 1, npc))

    # --- pass 1: greedy windows per core ---
    core_windows = []
    for k in range(N_CORES):
        e0, e1 = core_edge_start[k], core_edge_start[k + 1]
        dl = dst_s[e0:e1] - k * npc
        sl_lo = is_lo_node[src_s[e0:e1]]
        deg_lo = np.bincount(dl[sl_lo], minlength=npc)
        deg_hi = np.bincount(dl[~sl_lo], minlength=npc)
        wins = []
        d = 0
        while d < npc:
            start = d
            lo = hi = 0
            while (
                d < npc
                and d - start < 128
                and lo + deg_lo[d] <= EDGES_PER_HALF
                and hi + deg_hi[d] <= EDGES_PER_HALF
            ):
                lo += deg_lo[d]
                hi += deg_hi[d]
                d += 1
            assert d > start, "single dst exceeds per-window edge budget"
            wins.append((start, d))
        core_windows.append(wins)

    w_star = max(len(w) for w in core_windows)
    # O is only a compute-issue chunking of the window range now
    O = [0, (w_star + 1) // 2, w_star]
    W = (O[1], w_star - O[1])
    ls = w_star * 128
    rows = N_CORES * ls
    half_rows = rows // 2
    assert half_rows <= 32768 and rows - half_rows <= 32768
    for k in range(N_CORES):
        core_windows[k] = core_windows[k] + [(npc, npc)] * (
            w_star - len(core_windows[k])
        )

    # --- slots + rows for every node ---
    slot = np.full(n_nodes, -1, np.int64)
    for k in range(N_CORES):
        for c, (a, b) in enumerate(core_windows[k]):
            if b > a:
                d_loc = np.arange(a, b)
                slot[k * npc + d_loc] = c * 128 + (d_loc - a)
    assert (slot >= 0).all()
    owner = np.arange(n_nodes) // npc
    c_of = slot // 128
    p_of = slot % 128
    # owner-major rows: transpose convention fm pos s -> (p=s%128, c=s//128)
    row_of = owner * ls + p_of * w_star + c_of

    # gather op sizing: balance chunks over two queue rounds
    n_chunks = w_star * K_LO
    t_rounds = -(-n_chunks // 64)
    cpo = -(-n_chunks // (2 * t_rounds))
    assert cpo <= 32
    n_ops = -(-n_chunks // cpo)

    # --- per-core streams ---
    per_core = []
    for k in range(N_CORES):
        e0, e1 = core_edge_start[k], core_edge_start[k + 1]
        dl = dst_s[e0:e1] - k * npc
        sv = src_s[e0:e1]
        is_lo = is_lo_node[sv]
        idx_lo = np.zeros((w_star, EDGES_PER_HALF), np.int64)
        ids_lo = np.full((w_star, EDGES_PER_HALF), -1.0, np.float32)
        idx_hi = np.zeros_like(idx_lo)
        ids_hi = np.full_like(ids_lo, -1.0)
        # edges are dst-sorted; window edge groups are contiguous
        wbounds = np.searchsorted(dl, [a for a, _ in core_windows[k]] + [npc])
        for w, (a, b) in enumerate(core_windows[k]):
            lo_m = is_lo[wbounds[w] : wbounds[w + 1]]
            e_dst = dl[wbounds[w] : wbounds[w + 1]]
            e_src = sv[wbounds[w] : wbounds[w + 1]]
            for half, m in ((0, lo_m), (1, ~lo_m)):
                r = row_of[e_src[m]] - (0 if half == 0 else half_rows)
                cnt = r.shape[0]
                assert cnt <= EDGES_PER_HALF
                tgt_idx = idx_lo if half == 0 else idx_hi
                tgt_ids = ids_lo if half == 0 else ids_hi
                tgt_idx[w, :cnt] = r
                tgt_ids[w, :cnt] = (e_dst[m] - a).astype(np.float32)

        def _onehot(ids_arr):
            nch = ids_arr.size // 128
            ids_r = ids_arr.reshape(nch, 128)
            oh = (
                ids_r[:, :, None]
                == np.arange(128, dtype=np.float32)[None, None, :]
            )
            return np.ascontiguousarray(
                oh.transpose(1, 0, 2)
                .reshape(128, nch * 128)
                .astype(ml_dtypes.bfloat16)
            )

        per_core.append(
            dict(
                idx_lo=_wrap_idx(idx_lo.reshape(-1)),
                idx_hi=_wrap_idx(idx_hi.reshape(-1)),
                s_lo=_onehot(ids_lo.reshape(-1)),
                s_hi=_onehot(ids_hi.reshape(-1)),
            )
        )

    # --- x in both layouts ---
    x_bf = x.astype(ml_dtypes.bfloat16)
    x_full_nm = np.zeros((rows, F), ml_dtypes.bfloat16)
    x_full_nm[row_of] = x_bf

    in_maps = []
    for k in range(N_CORES):
        g = np.arange(k * npc, (k + 1) * npc)
        x_fm = np.zeros((F, ls), ml_dtypes.bfloat16)
        x_fm[:, slot[g]] = x_bf[g].T
        b_flat = np.full(ls, -1.0, np.float32)
        b_flat[slot[g]] = batch[g].astype(np.float32)
        batch_nm = b_flat.reshape(w_star, 128).T  # [p, c]
        b_onehot = (
            batch_nm[:, :, None] == np.arange(64, dtype=np.float32)[None, None, :]
        )
        b_onehot = np.ascontiguousarray(
            b_onehot.reshape(128, w_star * 64).astype(ml_dtypes.bfloat16)
        )
        m = dict(
            x_fm=x_fm,
            x_full_nm=x_full_nm,
            b_onehot=b_onehot,
            idx_lo=per_core[k]["idx_lo"],
            idx_hi=per_core[k]["idx_hi"],
            s_lo=per_core[k]["s_lo"],
            s_hi=per_core[k]["s_hi"],
            w1relT=np.ascontiguousarray(params["W1_rel"].T.astype(ml_dtypes.bfloat16)),
            w1rootT=np.ascontiguousarray(
                params["W1_root"].T.astype(ml_dtypes.bfloat16)
            ),
            w2relT=np.ascontiguousarray(params["W2_rel"].T.astype(ml_dtypes.bfloat16)),
            w2rootT=np.ascontiguousarray(
                params["W2_root"].T.astype(ml_dtypes.bfloat16)
            ),
            w3relT=np.ascontiguousarray(params["W3_rel"].T.astype(ml_dtypes.bfloat16)),
            w3rootT=np.ascontiguousarray(
                params["W3_root"].T.astype(ml_dtypes.bfloat16)
            ),
            b1=np.ascontiguousarray(params["b1_rel"].astype(np.float32).reshape(F, 1)),
            b2=np.ascontiguousarray(params["b2_rel"].astype(np.float32).reshape(F, 1)),
            b3=np.ascontiguousarray(params["b3_rel"].astype(np.float32).reshape(F, 1)),
            wlinT=np.ascontiguousarray(params["W_lin"].T.astype(np.float32)),
        )
        in_maps.append(m)

    meta = dict(
        w_star=w_star,
        ls=ls,
        rows=rows,
        half_rows=half_rows,
        n_graphs=n_graphs,
        W=tuple(W),
        O=tuple(O),
        cpo=cpo,
        n_ops=n_ops,
    )
    return meta, in_maps


def build_nc(meta, n_graphs_pad=64):
    w_star = meta["w_star"]
    ls = meta["ls"]
    rows = meta["rows"]
    half_rows = meta["half_rows"]
    W = meta["W"]
    O = meta["O"]
    CPO = meta["cpo"]
    n_ops = meta["n_ops"]
    sl_len = w_star * EDGES_PER_HALF  # idxs per stream
    n_chunks = sl_len // 128
    ng = n_graphs_pad

    nc = bacc.Bacc(
        "TRN2",
        target_bir_lowering=False,
        debug=False,
        num_devices=N_CORES,
        num_swdge_queues=4,
    )

    # --- I/O ---
    x_fm_d = nc.dram_tensor("x_fm", [F, ls], BF16, kind="ExternalInput")
    x_full_d = nc.dram_tensor("x_full_nm", [rows, F], BF16, kind="ExternalInput")
    bone_d = nc.dram_tensor("b_onehot", [128, w_star * 64], BF16, kind="ExternalInput")
    idx_d = {
        "lo": nc.dram_tensor("idx_lo", [128, sl_len // 16], I16, kind="ExternalInput"),
        "hi": nc.dram_tensor("idx_hi", [128, sl_len // 16], I16, kind="ExternalInput"),
    }
    s_d = {
        "lo": nc.dram_tensor("s_lo", [128, n_chunks * 128], BF16, kind="ExternalInput"),
        "hi": nc.dram_tensor("s_hi", [128, n_chunks * 128], BF16, kind="ExternalInput"),
    }
    w_d = {}
    for l in (1, 2, 3):
        for p in ("rel", "root"):
            w_d[l, p] = nc.dram_tensor(f"w{l}{p}T", [F, F], BF16, kind="ExternalInput")
    b_d = {l: nc.dram_tensor(f"b{l}", [F, 1], F32, kind="ExternalInput") for l in (1, 2, 3)}
    wlin_d = nc.dram_tensor("wlinT", [F, N_CLASSES], F32, kind="ExternalInput")
    out_d = nc.dram_tensor("out_partial", [N_CLASSES, ng], F32, kind="ExternalOutput")

    relu = mybir.ActivationFunctionType.Relu
    ident = mybir.ActivationFunctionType.Identity
    copy_f = mybir.ActivationFunctionType.Copy

    with tile.TileContext(nc) as tc:
        with (
            tc.tile_pool(name="const", bufs=1) as constp,
            tc.tile_pool(name="state", bufs=1) as statep,
            tc.tile_pool(name="gpool", bufs=4) as gpool,
            tc.tile_pool(name="spool", bufs=8) as spool,
            tc.tile_pool(name="psa", bufs=4, space="PSUM") as psa,
            tc.tile_pool(name="psd", bufs=2, space="PSUM") as psd,
            tc.tile_pool(name="psp", bufs=1, space="PSUM") as psp,
            tc.tile_pool(name="dram", bufs=1, space="DRAM") as dramp,
        ):
            nc.gpsimd.load_library(library_config.mlp)

            # ---- load constants ----
            bone_t = constp.tile([128, w_star * 64], BF16)
            nc.sync.dma_start(bone_t[:], bone_d[:])
            idx_t = {}
            for h in ("lo", "hi"):
                it = constp.tile([128, sl_len // 16], I16, name=f"idx_{h}")
                nc.sync.dma_start(it[:], idx_d[h][:])
                idx_t[h] = it
            w_t = {}
            for key, d in w_d.items():
                wt = constp.tile([F, F], BF16, name=f"w_{key[0]}_{key[1]}")
                nc.sync.dma_start(wt[:], d[:])
                w_t[key] = wt
            b_t = {}
            for l, d in b_d.items():
                bt = constp.tile([F, 1], F32, name=f"b_{l}")
                nc.sync.dma_start(bt[:], d[:])
                b_t[l] = bt
            wlin_t = constp.tile([F, N_CLASSES], F32)
            nc.sync.dma_start(wlin_t[:], wlin_d[:])

            x_fm_t = statep.tile([F, ls], BF16, tag="h0")
            nc.sync.dma_start(x_fm_t[:], x_fm_d[:])

            # ---- layers ----
            h_fm = x_fm_t
            gather_src = {
                "lo": x_full_d[0:half_rows, :],
                "hi": x_full_d[half_rows:rows, :],
            }
            pending_trigger = None
            for layer in (1, 2, 3):
                g_tiles = {"lo": [None] * n_ops, "hi": [None] * n_ops}
                s_tiles = {"lo": [None] * n_ops, "hi": [None] * n_ops}

                def issue_sload(h, o, layer=layer, s_tiles=s_tiles):
                    c0 = o * CPO
                    c1 = min(n_chunks, c0 + CPO)
                    nch = c1 - c0
                    st_ = spool.tile(
                        [128, nch * 128],
                        BF16,
                        name=f"sg_{layer}_{h}_{o}",
                        tag=f"sg_{h}",
                        bufs=4,
                    )
                    # s-loads ride the Sync HWDGE, all issued up-front so
                    # they precede the transposes in the HWDGE semaphore
                    # lane: Tile's DMA-lane sem targets are cumulative in
                    # program order, so an s-load issued after a transpose
                    # would drag all its consumers into waiting for it.
                    nc.sync.dma_start(st_[:], s_d[h][:, c0 * 128 : c1 * 128])
                    s_tiles[h][o] = st_

                def issue_gather(h, o, layer=layer, g_tiles=g_tiles,
                                 gather_src=gather_src):
                    c0 = o * CPO
                    c1 = min(n_chunks, c0 + CPO)
                    nch = c1 - c0
                    gt = gpool.tile(
                        [128, nch, F],
                        BF16,
                        name=f"g_{layer}_{h}_{o}",
                        tag=f"g_{h}",
                        padded_shape=[128, CPO, F],
                    )
                    nidx = nch * 128
                    # spread gathers over the 4 SWDGE queues: each queue is
                    # served by a dedicated Q7 core pair, so lo ops alternate
                    # pairs 0/1 and hi ops pairs 2/3.
                    qn = (o % 2) if h == "lo" else 2 + (o % 2)
                    nc.gpsimd.dma_gather(
                        gt[:],
                        gather_src[h],
                        idx_t[h][:, c0 * 8 : c1 * 8],
                        nidx,
                        nidx,
                        F,
                        single_packet=False,
                        queue_num=qn,
                    )
                    g_tiles[h][o] = gt

                agg_fm = statep.tile(
                    [F, ls], BF16, tag="agg", name=f"agg_{layer}"
                )
                h_next = statep.tile(
                    [F, ls], BF16, tag=f"h{layer % 2}", name=f"h_{layer}"
                )
                h_nm = statep.tile(
                    [128, w_star, F], BF16, tag="hnm", name=f"hnm_{layer}"
                )
                if layer < 3:
                    ag_in = dramp.tile(
                        [128, ls], BF16, name=f"agin_{layer}", tag=f"agin{layer % 2}"
                    )
                    hf = dramp.tile(
                        [rows, F],
                        BF16,
                        name=f"hf_{layer}",
                        tag=f"hf{layer}",
                        addr_space="Shared",
                    )

                s_issued = [0]

                def quarter_block(q, layer=layer, g_tiles=g_tiles, s_tiles=s_tiles,
                                  agg_fm=agg_fm, h_next=h_next, h_nm=h_nm,
                                  h_fm=h_fm, s_issued=s_issued):
                    for w in range(O[q], O[q + 1]):
                        ps = psa.tile(
                            [128, 128], F32, name=f"psagg_{layer}_{w}", tag="psagg"
                        )
                        for j in range(CHUNKS_PER_WIN):
                            h = "lo" if j < K_LO else "hi"
                            cc = w * K_LO + (j % K_LO)
                            o, sl_ = cc // CPO, cc % CPO
                            nc.tensor.matmul(
                                ps[:],
                                g_tiles[h][o][:, sl_, :],
                                s_tiles[h][o][:, sl_ * 128 : (sl_ + 1) * 128],
                                start=(j == 0),
                                stop=(j == CHUNKS_PER_WIN - 1),
                            )
                        nc.scalar.activation(
                            agg_fm[:, w * 128 : (w + 1) * 128], ps[:], copy_f
                        )

                def dense_block(q, layer=layer, agg_fm=agg_fm,
                                h_next=h_next, h_fm=h_fm):
                    # dense over this quarter's columns
                    c0_ = O[q] * 128
                    cq1 = O[q + 1] * 128
                    while c0_ < cq1:
                        cw = min(512, cq1 - c0_)
                        ps = psd.tile(
                            [128, 512], F32, name=f"psd_{layer}_{c0_}", tag="psd"
                        )
                        sl2 = slice(c0_, c0_ + cw)
                        nc.tensor.matmul(
                            ps[:, :cw],
                            w_t[layer, "rel"][:],
                            agg_fm[:, sl2],
                            start=True,
                            stop=False,
                        )
                        nc.tensor.matmul(
                            ps[:, :cw],
                            w_t[layer, "root"][:],
                            h_fm[:, sl2],
                            start=False,
                            stop=True,
                        )
                        nc.scalar.activation(
                            h_next[:, sl2],
                            ps[:, :cw],
                            relu if layer < 3 else ident,
                            bias=b_t[layer][:],
                        )
                        c0_ += cw

                def share_block(h_next=h_next, h_nm=h_nm,
                                ag_in=ag_in if layer < 3 else None,
                                layer=layer):
                    nc.sync.dma_start_transpose(h_nm[:], h_next[:])
                    if layer < 3:
                        nc.sync.dma_start(ag_in[:], h_nm[:])

                def issue_ag(ag_in=ag_in if layer < 3 else None,
                             hf=hf if layer < 3 else None):
                    nc.gpsimd.collective_compute(
                        "AllGather",
                        mybir.AluOpType.bypass,
                        replica_groups=[list(range(N_CORES))],
                        ins=[ag_in[:]],
                        outs=[hf[:]],
                    )

                # Issue sequence per layer:
                #   1. all gather ops (Pool stays pure; pacing via buffer
                #      rings) — s-loads ride Scalar at quarter boundaries;
                #   2. compute in two half-blocks so the lo half's dense +
                #      transpose + staging complete while the hi half still
                #      aggregates, letting AG-lo's transfer overlap it;
                #   3. AG triggers (Pool, after the gathers) fire as their
                #      staging lands; next layer's lo gathers only wait on
                #      AG-lo, so they start one transfer earlier than hi.
                for o in range(n_ops):
                    issue_sload("lo", o)
                    issue_sload("hi", o)
                    issue_gather("lo", o)
                    issue_gather("hi", o)
                quarter_block(0)
                dense_block(0)
                quarter_block(1)
                dense_block(1)
                share_block()
                if layer < 3:
                    issue_ag()

                if layer < 3:
                    gather_src = {
                        "lo": hf[0:half_rows, :],
                        "hi": hf[half_rows:rows, :],
                    }
                    h_fm = h_next
                else:
                    # pooling: pooledT[f, g] += h_nm[:, c, :].T @ B
                    ps_pool = psp.tile([128, ng], F32, tag="pspool")
                    for c in range(w_star):
                        nc.tensor.matmul(
                            ps_pool[:],
                            h_nm[:, c, :],
                            bone_t[:, c * ng : (c + 1) * ng],
                            start=(c == 0),
                            stop=(c == w_star - 1),
                        )
                    pooledT = statep.tile([128, ng], F32, tag="pooledT")
                    nc.scalar.activation(pooledT[:], ps_pool[:], copy_f)
                    ps_head = psp.tile([N_CLASSES, ng], F32, tag="pshead")
                    nc.tensor.matmul(ps_head[:], wlin_t[:], pooledT[:])
                    out_sb = statep.tile([N_CLASSES, ng], F32, tag="outsb")
                    nc.vector.tensor_copy(out_sb[:], ps_head[:])
                    nc.sync.dma_start(out_d[:], out_sb[:])

    nc.compile()
    return nc


def postprocess(results, batch, b_lin, n_graphs):
    """results: list of per-core dicts with 'out_partial' [10, ng]."""
    total = np.zeros_like(np.asarray(results[0]["out_partial"], np.float32))
    for r in results:
        total += np.asarray(r["out_partial"], np.float32)
    cnt = np.bincount(np.asarray(batch, np.int64), minlength=n_graphs).astype(
        np.float32
    )
    cnt = np.maximum(cnt, 1.0)
    logits = total[:, :n_graphs].T / cnt[:, None] + np.asarray(b_lin, np.float32)[None, :]
    return logits.astype(np.float32)


# ----------------------------------------------------------------------------
# harness entry point
# ----------------------------------------------------------------------------
from concourse.bass_utils import run_bass_kernel_spmd

_CACHE = {}


def kernel(x, edge_index, batch,
           W1_rel, b1_rel, W1_root,
           W2_rel, b2_rel, W2_root,
           W3_rel, b3_rel, W3_root,
           W_lin, b_lin):
    params = dict(W1_rel=W1_rel, b1_rel=b1_rel, W1_root=W1_root,
                  W2_rel=W2_rel, b2_rel=b2_rel, W2_root=W2_root,
                  W3_rel=W3_rel, b3_rel=b3_rel, W3_root=W3_root,
                  W_lin=W_lin, b_lin=b_lin)
    n_nodes = int(np.asarray(x).shape[0])
    n_graphs = 64
    meta, in_maps = preprocess(x, edge_index, batch, params, n_nodes, n_graphs)
    key = (meta["w_star"], meta["W"], meta["cpo"])
    if key not in _CACHE:
        _CACHE[key] = build_nc(meta)
    nc = _CACHE[key]
    res = run_bass_kernel_spmd(nc, in_maps, core_ids=list(range(N_CORES)))
    return postprocess(res.results, batch, b_lin, n_graphs)
